# revision 29
# baseline (speedup 1.0000x reference)
"""Trainium2 Bass kernel: attention-LSTM decoder (teacher-forced), 8 NeuronCores.

Strategy: the LSTM recurrence is the only sequential part. Everything else
(embedding @ W_ih, attention, fc1, fc2) is batched over all T steps.
Phases are replicated on all 8 cores; fc2 (the dominant matmul,
[B*T,H] @ [H,V]) is sharded over the vocab dim (V/8 = 4000 per core).
No collectives: the host concatenates the 8 logits shards.

v2: P = xe@W_ih^T+bias is computed INSIDE the recurrence loop (one step
ahead) directly into the PSUM banks that the gate matmuls then accumulate
into -- keeps the PE warm (no HAM re-throttle) and kills the DRAM round
trip. Attention/fc1/fc2 are pipelined per 8-batch group.

Layouts (device):
  gates column order is rearranged (host-side) so that each PSUM pass holds
  gate pairs interleaved per 256-wide h-window:
    pass0: [i | g] per window, pass1: [f | o] per window.
  PSUM partition p = 32*j + b  (j = h-window 0..3, b = batch 0..31)
  -> LSTM elementwise runs on all 128 partitions.
  h is transposed back each step (PE transpose) into
  HsT[hi, t, half, j, b]  (h-dim = 256*j + 128*half + hi).
"""

import numpy as np
import ml_dtypes

BF16 = ml_dtypes.bfloat16

V, E, H, B, T, S = 32000, 512, 1024, 32, 64, 64
NCORES = 8
VS = V // NCORES  # 4000 vocab cols per core
BT = B * T  # 2048


def _col_order():
    """Column permutation of the 4H gate dim used by W_ih/W_hh/bias on device."""
    order = []
    for p2 in range(2):
        ga = 0 if p2 == 0 else 1024      # i or f
        gb = 2048 if p2 == 0 else 3072   # g or o
        for j in range(4):
            order.extend(range(ga + j * 256, ga + (j + 1) * 256))
            order.extend(range(gb + j * 256, gb + (j + 1) * 256))
    return np.asarray(order, dtype=np.int64)


_NC = None


def _hsT(HsT, ko, t):
    """lhsT slice [128, 32] for contraction chunk ko of h_t."""
    return HsT[:, t, ko % 2, ko // 2, :]


def _phase01(nc, tc, dt, AF, xeT, wih, h0T, c0, sel4, biasP,
             whh_sb, ident_sb, HsT, preload):
    """Fused: P(t+1) precompute + LSTM recurrence step t."""
    with tc.tile_pool(name="xw", bufs=1) as xw, \
         tc.tile_pool(name="pps", bufs=6, space="PSUM") as pps, \
         tc.tile_pool(name="trps", bufs=2, space="PSUM") as trps, \
         tc.tile_pool(name="st1", bufs=2) as st1, \
         tc.tile_pool(name="ph1", bufs=1) as p1:
        xeT_sb = xw.tile([128, T, 4, 32], dt.bfloat16, tag="xeT")
        nc.sync.dma_start(xeT_sb[:], xeT[:])
        wih_sb = xw.tile([128, 4, 4096], dt.bfloat16, tag="wih")
        nc.sync.dma_start(wih_sb[:], wih[:])
        sel_sb = xw.tile([4, 128], dt.bfloat16, tag="sel4")
        nc.sync.dma_start(sel_sb[:], sel4[:])
        biasP_sb = xw.tile([4, 2, 512], dt.bfloat16, tag="biasP")
        nc.sync.dma_start(biasP_sb[:], biasP[:])

        h0T_sb = p1.tile([128, 8, 32], dt.bfloat16, tag="h0T")
        nc.sync.dma_start(h0T_sb[:], h0T[:])
        c_sb = p1.tile([128, 256], dt.float32, tag="c")
        nc.sync.dma_start(c_sb[:], c0[:])

        P_ps = {}

        def emit_P(t):
            tiles = []
            for p2 in range(2):
                ps = pps.tile([128, 512], dt.float32, tag="gates")
                for ko in range(4):
                    for j in range(4):
                        nc.tensor.matmul(
                            ps[32 * j:32 * (j + 1), :],
                            lhsT=xeT_sb[:, t, ko, :],
                            rhs=wih_sb[:, ko, (p2 * 4 + j) * 512:(p2 * 4 + j + 1) * 512],
                            start=(ko == 0), stop=False,
                            skip_group_check=True,
                            tile_position=(0, 32 * j),
                        )
                # + gate bias (selector matmul: partition group j gets row j)
                nc.tensor.matmul(ps[:, :], lhsT=sel_sb[:], rhs=biasP_sb[:, p2, :],
                                 start=False, stop=False, skip_group_check=True)
                tiles.append(ps)
            P_ps[t] = tiles

        emit_P(0)
        for t in range(T):
            if t == 6:
                # tail weights stream in behind the startup loads, while
                # W_hh/xeT are still alive (disjoint SBUF regions)
                preload()
            def hT(ko, _t=t):
                if _t == 0:
                    return h0T_sb[:, ko, :]
                return _hsT(HsT, ko, _t - 1)

            pspass = P_ps.pop(t)
            for p2 in range(2):
                ps = pspass[p2]
                for ko in range(8):
                    for j in range(4):
                        nc.tensor.matmul(
                            ps[32 * j:32 * (j + 1), :],
                            lhsT=hT(ko),
                            rhs=whh_sb[:, ko, (p2 * 4 + j) * 512:(p2 * 4 + j + 1) * 512],
                            start=False, stop=(ko == 7),
                            skip_group_check=True,
                            tile_position=(0, 32 * j),
                        )

            # next step's P runs on PE while this step's elementwise happens
            if t + 1 < T:
                emit_P(t + 1)

            ig_sb = st1.tile([128, 512], dt.bfloat16, tag="ig")
            nc.scalar.activation(ig_sb[:, 0:256], pspass[0][:, 0:256], AF.Sigmoid)
            nc.scalar.activation(ig_sb[:, 256:512], pspass[0][:, 256:512], AF.Tanh)
            ig2 = st1.tile([128, 256], dt.float32, tag="ig2")
            nc.vector.tensor_mul(ig2[:], ig_sb[:, 0:256], ig_sb[:, 256:512])

            # f/o + cell update, pipelined per 128-wide half to shorten the
            # serial chain into the next step's matmuls
            fo_sb = st1.tile([128, 512], dt.bfloat16, tag="fo")
            fc_ = st1.tile([128, 256], dt.float32, tag="fc")
            thc = st1.tile([128, 256], dt.float32, tag="thc")
            h_sb = st1.tile([128, 256], dt.bfloat16, tag="h")
            for half in range(2):
                hs = slice(half * 128, (half + 1) * 128)
                os_ = slice(256 + half * 128, 384 + half * 128)
                nc.scalar.activation(fo_sb[:, hs], pspass[1][:, hs], AF.Sigmoid)
                nc.vector.tensor_mul(fc_[:, hs], fo_sb[:, hs], c_sb[:, hs])
                nc.vector.tensor_add(c_sb[:, hs], fc_[:, hs], ig2[:, hs])
                nc.scalar.activation(thc[:, hs], c_sb[:, hs], AF.Tanh)
                nc.scalar.activation(fo_sb[:, os_], pspass[1][:, os_], AF.Sigmoid)
                nc.vector.tensor_mul(h_sb[:, hs], fo_sb[:, os_], thc[:, hs])
                trp = trps.tile([128, 128], dt.bfloat16, tag="tr")
                nc.tensor.transpose(trp[:], h_sb[:, hs], ident_sb[:])
                nc.vector.tensor_copy(
                    HsT[:, t, half, :, :],
                    trp[:].rearrange("p (j b) -> p j b", j=4),
                )


def _tail(nc, tc, dt, AF, mybir, encT, encsp, fc2w, fc2b, out, HsT, ident_sb,
          fc1w_sb, fc1b_sb, ones_sb):
    """Attention + fc1 + fc2, software-pipelined one batch-group ahead."""
    with tc.tile_pool(name="tw", bufs=1) as tw, \
         tc.tile_pool(name="ztp", bufs=2) as ztp, \
         tc.tile_pool(name="ebg", bufs=2) as ebg, \
         tc.tile_pool(name="esp", bufs=2) as esp, \
         tc.tile_pool(name="ctxbg", bufs=2) as cbg, \
         tc.tile_pool(name="wvp", bufs=2) as wvp, \
         tc.tile_pool(name="ast", bufs=3) as ast, \
         tc.tile_pool(name="ost", bufs=4) as ost, \
         tc.tile_pool(name="scps", bufs=2, space="PSUM") as scps, \
         tc.tile_pool(name="atps", bufs=1, space="PSUM") as atps, \
         tc.tile_pool(name="ctps", bufs=1, space="PSUM") as ctps, \
         tc.tile_pool(name="f1ps", bufs=2, space="PSUM") as f1ps, \
         tc.tile_pool(name="f2ps", bufs=2, space="PSUM") as f2ps:
        fc2b_sb = tw.tile([1, VS], dt.bfloat16, tag="fc2b")
        nc.sync.dma_start(fc2b_sb[:], fc2b[:])

        ctx_tiles = {}

        def attn(bg):
            bsl = slice(bg * 8, (bg + 1) * 8)
            encT_bg = ebg.tile([128, 8, 8, 64], dt.bfloat16, tag="encT")
            nc.sync.dma_start(encT_bg[:], encT[bg])
            esp_bg = esp.tile([64, 8, 8, 128], dt.bfloat16, tag="esp")
            nc.sync.dma_start(esp_bg[:], encsp[:, bsl, :, :])

            ps_sc = scps.tile([64, 512], dt.float32, tag="sc")
            for bi in range(8):
                b = bg * 8 + bi
                for ko in range(8):
                    nc.tensor.matmul(
                        ps_sc[:, bi * 64:(bi + 1) * 64],
                        lhsT=HsT[:, :, ko % 2, ko // 2, b],
                        rhs=encT_bg[:, ko, bi, :],
                        start=(ko == 0), stop=(ko == 7),
                        skip_group_check=True,
                    )
            mx = ast.tile([64, 8], dt.float32, tag="mx")
            nc.vector.reduce_max(mx[:], ps_sc[:].rearrange("p (b s) -> p b s", s=64),
                                 axis=mybir.AxisListType.X)
            sc2 = ast.tile([64, 8, 64], dt.float32, tag="sc2")
            nc.vector.tensor_sub(sc2[:], ps_sc[:].rearrange("p (b s) -> p b s", s=64),
                                 mx[:, :, None].to_broadcast((64, 8, 64)))
            nc.scalar.activation(sc2[:], sc2[:], AF.Exp)
            sm = ast.tile([64, 8], dt.float32, tag="sm")
            nc.vector.reduce_sum(sm[:], sc2[:], axis=mybir.AxisListType.X)
            nc.vector.reciprocal(sm[:], sm[:])
            a_sb = ast.tile([64, 8, 64], dt.bfloat16, tag="a")
            nc.vector.tensor_mul(a_sb[:], sc2[:],
                                 sm[:, :, None].to_broadcast((64, 8, 64)))

            # ctx layout [hi, ho, t, bi]: fc1 consumes (t, b) order
            ctxT_bg = cbg.tile([128, 8, 64, 8], dt.bfloat16, tag="ctx")
            ctx_tiles[bg] = ctxT_bg
            for bi in range(8):
                b = bg * 8 + bi
                psT = atps.tile([64, 64], dt.bfloat16, tag="aT")
                nc.tensor.transpose(psT[:], a_sb[:, bi, :], ident_sb[0:64, 0:64])
                aT_sb = ast.tile([64, 64], dt.bfloat16, tag="aTs")
                nc.vector.tensor_copy(aT_sb[:], psT[:])
                ps_ctx = ctps.tile([128, 512], dt.float32, tag="ctx")
                for ho in range(8):
                    nc.tensor.matmul(
                        ps_ctx[:, ho * 64:(ho + 1) * 64],
                        lhsT=esp_bg[:, bi, ho, :],
                        rhs=aT_sb[:],
                        start=True, stop=True,
                        skip_group_check=True,
                    )
                nc.vector.tensor_copy(
                    ctxT_bg[:, :, :, bi],
                    ps_ctx[:].rearrange("p (ho t) -> p ho t", t=64),
                )

        def fc12(bg):
            bsl = slice(bg * 8, (bg + 1) * 8)
            ctxT_bg = ctx_tiles.pop(bg)
            ZTb = ztp.tile([128, 8, 512], dt.bfloat16, tag="ZT")
            for mo in range(8):
                ps = f1ps.tile([128, 512], dt.float32, tag="ps")
                for ko in range(16):
                    if ko < 8:
                        rhs = HsT[:, :, ko % 2, ko // 2, bsl]
                    else:
                        rhs = ctxT_bg[:, ko - 8, :, :]
                    nc.tensor.matmul(
                        ps[:],
                        lhsT=fc1w_sb[:, ko, mo * 128:(mo + 1) * 128],
                        rhs=rhs,
                        start=(ko == 0), stop=(ko == 15),
                        skip_group_check=True,
                    )
                nc.scalar.activation(ZTb[:, mo, :], ps[:],
                                     AF.Tanh, bias=fc1b_sb[:, mo:mo + 1])

            for vo in range(8):
                wv = wvp.tile([128, 8, 500], dt.bfloat16, tag="wv")
                nc.sync.dma_start(wv[:], fc2w[vo])
                for mi in range(4):
                    mo = bg * 4 + mi
                    ps = f2ps.tile([128, 500], dt.float32, tag="ps")
                    for ko in range(8):
                        nc.tensor.matmul(
                            ps[:],
                            lhsT=ZTb[:, ko, mi * 128:(mi + 1) * 128],
                            rhs=wv[:, ko, :],
                            start=(ko == 0), stop=False,
                            skip_group_check=True,
                        )
                    nc.tensor.matmul(ps[:], lhsT=ones_sb[:],
                                     rhs=fc2b_sb[:, vo * 500:(vo + 1) * 500],
                                     start=False, stop=True, skip_group_check=True)
                    ob = ost.tile([128, 500], dt.float32, tag="ob")
                    nc.vector.tensor_copy(ob[:], ps[:])
                    nc.sync.dma_start(
                        out[mo * 128:(mo + 1) * 128, vo * 500:(vo + 1) * 500], ob[:])

        # pipeline: attention runs one bg ahead of fc1/fc2
        attn(0)
        attn(1)
        fc12(0)
        attn(2)
        fc12(1)
        attn(3)
        fc12(2)
        fc12(3)


def _build():
    """Build the Bass graph (single NeuronCore program, SPMD across 8)."""
    import concourse.mybir as mybir
    from concourse import bacc
    import concourse.tile as tile

    dt = mybir.dt
    AF = mybir.ActivationFunctionType

    nc = bacc.Bacc(None, target_bir_lowering=False)

    def inp(name, shape, dtp):
        return nc.declare_dram_parameter(name, list(shape), dtp, isOutput=False)

    xeT = inp("xeT", (128, T, 4, 32), dt.bfloat16)       # emb[inputs] transposed
    wih = inp("wih", (128, 4, 4096), dt.bfloat16)        # W_ih^T, arranged cols
    whh = inp("whh", (128, 8, 4096), dt.bfloat16)        # W_hh^T, arranged cols
    sel4 = inp("sel4", (4, 128), dt.bfloat16)            # bias selector
    biasP = inp("biasP", (4, 2, 512), dt.bfloat16)       # (b_ih+b_hh) arranged
    ident = inp("ident", (128, 128), dt.bfloat16)
    ones1 = inp("ones1", (1, 128), dt.bfloat16)
    encT = inp("encT", (4, 128, 8, 8, 64), dt.bfloat16)  # enc[h,b,s], bg-major
    encsp = inp("encsp", (64, 32, 8, 128), dt.bfloat16)  # enc[s,b,ho,hi] s-part
    h0T = inp("h0T", (128, 8, 32), dt.bfloat16)
    c0 = inp("c0", (128, 256), dt.float32)               # cell, (j,b) layout
    fc1w = inp("fc1w", (128, 16, 1024), dt.bfloat16)     # fc1_W^T
    fc1b = inp("fc1b", (128, 8), dt.float32)
    fc2w = inp("fc2w", (8, 128, 8, 500), dt.bfloat16)    # per-core V slice, vo-major
    fc2b = inp("fc2b", (1, VS), dt.bfloat16)
    out = nc.declare_dram_parameter("out", [BT, VS], dt.float32, isOutput=True)

    with tile.TileContext(nc) as tc:
        with tc.tile_pool(name="persist", bufs=1) as pp, \
             tc.tile_pool(name="hstp", bufs=1) as hstp, \
             tc.tile_pool(name="twE", bufs=1) as twE:
            ident_sb = pp.tile([128, 128], dt.bfloat16, tag="ident")
            nc.sync.dma_start(ident_sb[:], ident[:])
            HsT = hstp.tile([128, T, 2, 4, 32], dt.bfloat16, tag="HsT")

            fc1w_sb = twE.tile([128, 16, 1024], dt.bfloat16, tag="fc1w")
            fc1b_sb = twE.tile([128, 8], dt.float32, tag="fc1b")
            ones_sb = twE.tile([1, 128], dt.bfloat16, tag="ones1")

            def preload():
                nc.sync.dma_start(fc1w_sb[:], fc1w[:])
                nc.sync.dma_start(fc1b_sb[:], fc1b[:])
                nc.sync.dma_start(ones_sb[:], ones1[:])

            with tc.tile_pool(name="whhp", bufs=1) as whp:
                whh_sb = whp.tile([128, 8, 4096], dt.bfloat16, tag="whh")
                nc.sync.dma_start(whh_sb[:], whh[:])
                _phase01(nc, tc, dt, AF, xeT, wih, h0T, c0, sel4, biasP,
                         whh_sb, ident_sb, HsT, preload)

            _tail(nc, tc, dt, AF, mybir, encT, encsp, fc2w, fc2b, out, HsT,
                  ident_sb, fc1w_sb, fc1b_sb, ones_sb)

    nc.compile()
    return nc


def _get_nc():
    global _NC
    if _NC is None:
        _NC = _build()
    return _NC


def _prep_inputs(inputs, hiddens, hidden, cell, emb, W_ih, b_ih, W_hh, b_hh,
                 fc1_W, fc1_b, fc2_W, fc2_b):
    """Host-side layout prep (gather / transpose / cast only)."""
    order = _col_order()
    f32 = np.float32

    inds = np.asarray(inputs).astype(np.int64)
    xe = np.asarray(emb, f32)[inds]                      # [B, T, E]
    xeT = np.ascontiguousarray(
        xe.reshape(B, T, 4, 128).transpose(3, 1, 2, 0)).astype(BF16)

    wih_a = np.ascontiguousarray(
        np.asarray(W_ih, f32).T[:, order].reshape(4, 128, 4096)
        .transpose(1, 0, 2)).astype(BF16)
    whh_a = np.ascontiguousarray(
        np.asarray(W_hh, f32).T[:, order].reshape(8, 128, 4096)
        .transpose(1, 0, 2)).astype(BF16)

    bias_vec = (np.asarray(b_ih, f32) + np.asarray(b_hh, f32))[order]
    biasP = np.ascontiguousarray(
        bias_vec.reshape(2, 4, 512).transpose(1, 0, 2)).astype(BF16)  # [4,2,512]
    sel4 = np.repeat(np.eye(4, dtype=f32), 32, axis=1).astype(BF16)   # [4,128]

    ident = np.eye(128, dtype=f32).astype(BF16)
    ones1 = np.ones((1, 128), f32).astype(BF16)

    hid = np.asarray(hiddens, f32)                       # [S, B, H]
    # encT[bg, ki, ko, bi, s] = hid[s, bg*8+bi, ko*128+ki]
    encT = np.ascontiguousarray(
        hid.reshape(S, 4, 8, 8, 128).transpose(1, 4, 3, 2, 0)).astype(BF16)
    encsp = np.ascontiguousarray(hid.reshape(S, B, 8, 128)).astype(BF16)

    h0T = np.ascontiguousarray(
        np.asarray(hidden, f32).reshape(B, 8, 128).transpose(2, 1, 0)).astype(BF16)
    c0a = np.ascontiguousarray(
        np.asarray(cell, f32).reshape(B, 4, 256).transpose(1, 0, 2).reshape(128, 256))

    fc1w_a = np.ascontiguousarray(
        np.asarray(fc1_W, f32).T.reshape(16, 128, 1024).transpose(1, 0, 2)).astype(BF16)
    fc1b_a = np.ascontiguousarray(np.asarray(fc1_b, f32).reshape(8, 128).T)

    common = dict(xeT=xeT, wih=wih_a, whh=whh_a, sel4=sel4, biasP=biasP,
                  ident=ident, ones1=ones1, encT=encT, encsp=encsp, h0T=h0T,
                  c0=c0a, fc1w=fc1w_a, fc1b=fc1b_a)

    fc2_W = np.asarray(fc2_W, f32)
    fc2_b = np.asarray(fc2_b, f32)
    in_maps = []
    for r in range(NCORES):
        sl = slice(r * VS, (r + 1) * VS)
        fc2w_r = np.ascontiguousarray(
            fc2_W[sl].T.reshape(8, 128, 8, 500).transpose(2, 1, 0, 3)).astype(BF16)
        fc2b_r = np.ascontiguousarray(fc2_b[sl].reshape(1, VS)).astype(BF16)
        in_maps.append({**common, "fc2w": fc2w_r, "fc2b": fc2b_r})
    return in_maps


def kernel(inputs, hiddens, hidden, cell, emb, W_ih, b_ih, W_hh, b_hh,
           fc1_W, fc1_b, fc2_W, fc2_b, generate_len=None, _trace=False,
           _tmpdir=None):
    from concourse.bass_utils import run_bass_kernel_spmd

    in_maps = _prep_inputs(inputs, hiddens, hidden, cell, emb, W_ih, b_ih,
                           W_hh, b_hh, fc1_W, fc1_b, fc2_W, fc2_b)
    nc = _get_nc()
    res = run_bass_kernel_spmd(nc, in_maps, list(range(NCORES)),
                               trace=_trace, tmpdir=_tmpdir)
    shards = [np.asarray(res.results[r]["out"], np.float32) for r in range(NCORES)]
    full = np.concatenate(shards, axis=1)
    # rows are (bg, mo4, t_loc, b_loc): b = bg*8+b_loc, t = mo4*16+t_loc
    out = np.ascontiguousarray(
        full.reshape(4, 4, 16, 8, V).transpose(0, 3, 1, 2, 4).reshape(B, T, V))
    if _trace:
        return out, res
    return out
